# revision 11
# baseline (speedup 1.0000x reference)
"""Trainium2 Bass kernel for nn_CNOLReLu: bicubic 2x upsample -> leaky_relu
-> antialiased bicubic 2x downsample on (16,128,128,128) NHWC f32.

Data-parallel over batch: 2 images/core.  Per channel (X = x[b,:,:,c]):
  Y = D @ lrelu(U X U^T) @ D^T,  U: 128->256 bicubic, D: 256->128 antialiased.
Hop chain per channel (ds = data-stationary, ms = matrix-stationary):
  h1 ds: T1[w,h2]    = X^T  @ U^T            (N=256)
  h2 ms: Zt[w2t,h2]  = Uw_t @ T1             (N=512, 2 chans/MM)
  f:     ScalarE Lrelu on PSUM->SBUF evac
  h3 ds: S[h2m,w']  += R_t[:,h2m]^T @ Dw_t   (banded N=66, accum over t)
  h4 ms: Y[h',w']    = sum_m Dh_m^T @ S_m    (N=512, 4 chans/MM)
Host pre/post transposes give contiguous SBUF slices everywhere:
  x DRAM layout [b,h,c,w], y DRAM layout [b,h,c,w] (=> transpose back).
"""
import numpy as np
import ml_dtypes
from contextlib import ExitStack

import concourse.bacc as bacc
import concourse.tile as tile
from concourse import mybir
from concourse.bass_utils import run_bass_kernel_spmd

F32 = mybir.dt.float32
BF16 = mybir.dt.bfloat16
AF = mybir.ActivationFunctionType

N_CORES = 8
B_CORE = 2          # images per core
H = W = C = 128
NEG_SLOPE = 0.01


def _keys_cubic(x):
    x = np.abs(x)
    return np.where(
        x <= 1, (1.5 * x - 2.5) * x * x + 1,
        np.where(x < 2, ((-0.5 * x + 2.5) * x - 4) * x + 2, 0.0))


def _resize_matrix(n_in, n_out):
    """Row-stochastic bicubic (antialias) resize operator; matches
    jax.image.resize(method='bicubic', antialias=True)."""
    scale = n_out / n_in
    pos = (np.arange(n_out) + 0.5) / scale - 0.5
    kscale = min(scale, 1.0)
    w = _keys_cubic((np.arange(n_in)[None, :] - pos[:, None]) * kscale)
    return (w / w.sum(axis=1, keepdims=True)).astype(np.float64)


def _band(Dm, t):
    rows = np.nonzero(np.abs(Dm[:, t * 128:(t + 1) * 128]).sum(1) > 0)[0]
    return int(rows.min()), int(rows.max()) + 1


_CACHE = {}


def _build(repeat=1):
    if repeat in _CACHE:
        return _CACHE[repeat]

    U = _resize_matrix(H, 2 * H)       # [256,128]
    Dm = _resize_matrix(2 * H, H)      # [128,256]
    uT = U.T.astype(ml_dtypes.bfloat16)                              # [128,256]
    dT = np.concatenate([Dm.T[0:128, :], Dm.T[128:256, :]], axis=1)  # [128,256]
    dT_bf = dT.astype(ml_dtypes.bfloat16)
    bands = [_band(Dm, 0), _band(Dm, 1)]   # [(0,66),(62,128)]

    nc = bacc.Bacc()
    x_d = nc.declare_dram_parameter("x", [B_CORE, H, C, W], BF16, isOutput=False)
    ut_d = nc.declare_dram_parameter("ut", [128, 256], BF16, isOutput=False)
    dbf_d = nc.declare_dram_parameter("dbf", [128, 256], BF16, isOutput=False)
    y_d = nc.declare_dram_parameter("y", [B_CORE, H, C, W], BF16, isOutput=True)

    with tile.TileContext(nc) as tc, ExitStack() as ctx:
        wpool = ctx.enter_context(tc.tile_pool(name="weights", bufs=1))
        xpool = ctx.enter_context(tc.tile_pool(name="ximg", bufs=2))
        opool = ctx.enter_context(tc.tile_pool(name="oimg", bufs=2))
        spool = ctx.enter_context(tc.tile_pool(name="stage", bufs=2))
        ppool = ctx.enter_context(tc.tile_pool(name="psum", bufs=1, space="PSUM"))

        ut_s = wpool.tile([128, 256], BF16, tag="ut")
        dbf_s = wpool.tile([128, 256], BF16, tag="dbf")
        nc.sync.dma_start(ut_s[:], ut_d[:])
        nc.sync.dma_start(dbf_s[:], dbf_d[:])

        def h1(ximg, blk, p):
            # T1[w, h2] per chan; pair -> pT1 [128, 2*256]
            pT1 = ppool.tile([128, 512], F32, tag="pT1", bufs=2)
            for j in range(2):
                c = blk * 4 + p * 2 + j
                nc.tensor.matmul(pT1[:, j * 256:(j + 1) * 256],
                                 ximg[:, c * 128:(c + 1) * 128],
                                 ut_s[:], start=True, stop=True)
            sT1 = spool.tile([128, 512], BF16, tag="sT1")
            nc.vector.tensor_copy(sT1[:], pT1[:])
            return sT1

        def h2(sT1, sR, p):
            # Zt[w2t, (cl h2)] = Uw_t @ T1; lrelu evac by ScalarE
            pZ = ppool.tile([128, 1024], F32, tag="pZ", bufs=2)
            for t in range(2):
                nc.tensor.matmul(pZ[:, t * 512:(t + 1) * 512],
                                 ut_s[:, t * 128:(t + 1) * 128],
                                 sT1[:], start=True, stop=True)
            # sR cols = p*1024 + t*512 + cl*256 + h2
            nc.scalar.activation(sR[:, p * 1024:(p + 1) * 1024],
                                 pZ[:], AF.Lrelu, alpha=NEG_SLOPE)

        def h3(sR, sS, p):
            # banded W-down; pS cols = cl*256 + m*128 + w'
            pS = ppool.tile([128, 512], F32, tag="pS", bufs=1)
            for cl in range(2):
                for m in range(2):
                    for t in range(2):
                        lo, hi = bands[t]
                        nc.tensor.matmul(
                            pS[:, cl * 256 + m * 128 + lo:
                               cl * 256 + m * 128 + hi],
                            sR[:, p * 1024 + t * 512 + cl * 256 +
                               m * 128: p * 1024 + t * 512 + cl * 256 +
                               (m + 1) * 128],
                            dbf_s[:, t * 128 + lo:t * 128 + hi],
                            start=(t == 0), stop=(t == 1),
                            skip_group_check=True)
            # evac pS -> sS cols m*512 + (p*2+cl)*128 + w'
            src = pS[:].rearrange("h (c m w) -> h m c w", c=2, m=2, w=128)
            dst = sS[:].rearrange("h (m c w) -> h m c w", m=2, c=4,
                                  w=128)[:, :, p * 2:(p + 1) * 2, :]
            nc.vector.tensor_copy(dst, src)

        def h4_mm(blk, sS, oimg, bdone):
            # pY[h', (c w')] = sum_m Dh_m^T @ sS_m
            pY = ppool.tile([128, 512], F32, tag="pY", bufs=1)
            for m in range(2):
                nc.tensor.matmul(pY[:],
                                 dbf_s[:, m * 128:(m + 1) * 128],
                                 sS[:, m * 512:(m + 1) * 512],
                                 start=(m == 0), stop=(m == 1))
            return pY

        def h4_evac(pY, blk, sS, oimg, bdone):
            nc.scalar.copy(oimg[:, blk * 512:(blk + 1) * 512], pY[:])
            if bdone is not None:
                nc.sync.dma_start(bdone.rearrange("h c w -> h (c w)"), oimg[:])

        # software pipeline: h4/pY of block k-1 run mid-block k, carried
        # across image boundaries (bdone = DRAM dest once image complete)
        prev = None                    # (blk, sS, oimg, bdone)
        ximg = oimg = None
        NB = C // 4
        for b in [ib for _ in range(repeat) for ib in range(B_CORE)]:
            ximg = xpool.tile([128, C * W], BF16, tag="ximg")
            nc.sync.dma_start(ximg[:], x_d[b].rearrange("h c w -> h (c w)"))
            oimg = opool.tile([128, C * W], BF16, tag="oimg")
            for blk in range(NB):
                sR = spool.tile([128, 2048], BF16, tag="sR")
                sS = spool.tile([128, 1024], BF16, tag="sS")
                sT1a = h1(ximg, blk, 0)
                sT1b = h1(ximg, blk, 1)
                h2(sT1a, sR, 0)
                if prev is not None:
                    pY = h4_mm(*prev)
                h3(sR, sS, 0)
                h2(sT1b, sR, 1)
                if prev is not None:
                    h4_evac(pY, *prev)
                h3(sR, sS, 1)
                prev = (blk, sS, oimg,
                        y_d[b] if blk == NB - 1 else None)
        h4_evac(h4_mm(*prev), *prev)

    nc.compile()
    consts = {"ut": np.ascontiguousarray(uT),
              "dbf": np.ascontiguousarray(dT_bf)}
    _CACHE[repeat] = (nc, consts)
    return nc, consts


def prepare(x, repeat=1):
    x = np.asarray(x, dtype=np.float32)
    assert x.shape == (16, H, W, C), x.shape
    xt = np.ascontiguousarray(x.transpose(0, 1, 3, 2))  # [b, h, c, w]
    nc, consts = _build(repeat)
    in_maps = []
    for core in range(N_CORES):
        m = {"x": xt[core * B_CORE:(core + 1) * B_CORE].astype(
            ml_dtypes.bfloat16)}
        m.update(consts)
        in_maps.append(m)
    return nc, in_maps


def kernel(x, in_size=128, out_size=128, trace=False, tmpdir=None):
    nc, in_maps = prepare(x)
    res = run_bass_kernel_spmd(nc, in_maps, list(range(N_CORES)), trace=trace,
                               tmpdir=tmpdir)
    out = np.concatenate([res.results[i]["y"] for i in range(N_CORES)], axis=0)
    if trace:
        kernel.last_exec_time_ns = res.exec_time_ns
        kernel.last_results = res
    # y DRAM layout is [b, h, c, w] -> NHWC
    return out.astype(np.float32).transpose(0, 1, 3, 2)
